# revision 1
# baseline (speedup 1.0000x reference)
"""Trainium2 Bass kernel for nn_CrossAttentionGraphBlock.

Strategy (hardcoded for B=16, NQ=512, NK=1024, D=768, L=512, H=12, DH=64):
 - Data-parallel over batch: 8 cores x 2 batches each. No collectives.
 - Host-side (numpy, cheap): fuse the outer q/k/v projections with the
   in-proj of MultiheadAttention (both are linear):
       qWe = qW @ in_qW / sqrt(DH)   (softmax scale folded in)
       kWe = kW @ in_kW,  vWe = vW @ in_vW  (+ fused biases)
   and pre-transpose activations so the device never transposes inputs.
 - On-chip dataflow is feature-major ([feature, token] in SBUF) end to end:
   projections, attention, out-proj, both layernorms.  Per head h:
       S^T[k,q]  = (kh_h)^T.T @ qh_h^T          (K=DH=64 contraction)
       P^T       = exp(S^T)                      (no max-sub needed: |S|<~1)
       ctx~aug^T = vh_aug.T @ P^T                (65th row = softmax denom)
   then ctx^T rows are scaled by 1/denom (PE broadcast of the reciprocal).
   Key-padding mask is applied by zeroing masked rows of vh_aug (incl. the
   ones-column), which removes masked keys from both ctx and the denom.
 - Heads are processed in pairs with interleaved S matmuls at partition
   bases 0/64 -> tile_position row groups (0,0)/(64,0) run concurrently.
 - LayerNorm stats across features (= partitions) via ones-column matmuls.
 - Final [feat,tok] -> [tok,feat] transpose on the tensor engine.
 - Precision: matmuls bf16 (fp32 PSUM accumulate); LN stats, softmax
   denominators and the final LN2 output stay fp32.
"""

import numpy as np
import ml_dtypes

import concourse.bass as bass
import concourse.mybir as mybir
import concourse.tile as tile
from concourse import bacc
from concourse.bass_utils import run_bass_kernel_spmd

P = 128
B, NQ, NK, D, L, H = 16, 512, 1024, 768, 512, 12
DH = D // H          # 64
NCORES = 8
BL = B // NCORES     # 2 batches per core
EPS = 1e-5
F32 = mybir.dt.float32
BF16 = mybir.dt.bfloat16
NPBF = ml_dtypes.bfloat16
AF = mybir.ActivationFunctionType
ALU = mybir.AluOpType

KD = D // P          # 6 chunks of the D (=768) contraction/feature dim
KL = L // P          # 4 chunks of the L (=512) contraction dim
MQ = NQ // P         # 4 query-token tiles
MK = NK // P         # 8 key-token tiles
VA = H * (DH + 1)    # 780: vh augmented with one ones-column per head

_NC_CACHE = {}


def _build_nc():
    nc = bacc.Bacc()

    gnT = nc.declare_dram_parameter("gnT", [BL, D, NQ], BF16, isOutput=False)
    gnTf = nc.declare_dram_parameter("gnTf", [BL, D, NQ], F32, isOutput=False)
    condT = nc.declare_dram_parameter("condT", [BL, L, NK], BF16, isOutput=False)
    qWe = nc.declare_dram_parameter("qWe", [D, D], BF16, isOutput=False)
    kWe = nc.declare_dram_parameter("kWe", [L, D], BF16, isOutput=False)
    vWe = nc.declare_dram_parameter("vWe", [L, VA], BF16, isOutput=False)
    outW = nc.declare_dram_parameter("outW", [D, D], BF16, isOutput=False)
    d1W = nc.declare_dram_parameter("d1W", [D, D], BF16, isOutput=False)
    bcols = nc.declare_dram_parameter("bcols", [P, 8 * KD], F32, isOutput=False)
    vber = nc.declare_dram_parameter("vber", [1, VA], BF16, isOutput=False)
    vld = nc.declare_dram_parameter("vld", [P, BL * MK], F32, isOutput=False)
    onesr = nc.declare_dram_parameter("onesr", [1, P], F32, isOutput=False)
    onesrb = nc.declare_dram_parameter("onesrb", [1, P], BF16, isOutput=False)
    onescb = nc.declare_dram_parameter("onescb", [P, 1], BF16, isOutput=False)
    onescf = nc.declare_dram_parameter("onescf", [P, 1], F32, isOutput=False)
    ident = nc.declare_dram_parameter("ident", [P, P], F32, isOutput=False)
    out = nc.declare_dram_parameter("out", [BL, NQ, D], F32, isOutput=True)

    with tile.TileContext(nc) as tc:
        with (
            tc.tile_pool(name="const", bufs=1) as cst,
            tc.tile_pool(name="gnT", bufs=2) as gnp,
            tc.tile_pool(name="big", bufs=2) as big,      # condT / per-head P~ / LN sq
            tc.tile_pool(name="kh", bufs=2) as khp,
            tc.tile_pool(name="qh", bufs=2) as qhp,
            tc.tile_pool(name="vh", bufs=2) as vhp,
            tc.tile_pool(name="xres", bufs=1) as xrp,
            tc.tile_pool(name="yy", bufs=1) as yyp,
            tc.tile_pool(name="outp", bufs=1) as otp,
            tc.tile_pool(name="ctx", bufs=1) as ctp,
            tc.tile_pool(name="zz", bufs=2) as zzp,
            tc.tile_pool(name="small", bufs=1) as sml,
            tc.tile_pool(name="sp", bufs=3, space="PSUM") as spp,   # [P,1024] 2-bank
            tc.tile_pool(name="mm", bufs=2, space="PSUM") as mmp,   # [P,512]
        ):
            # ---- resident constants -------------------------------------
            qWe_sb = cst.tile([P, KD, D], BF16, tag="qWe")
            nc.sync.dma_start(qWe_sb[:], qWe[:].rearrange("(ko p) n -> p ko n", p=P))
            kWe_sb = cst.tile([P, KL, D], BF16, tag="kWe")
            nc.sync.dma_start(kWe_sb[:], kWe[:].rearrange("(ko p) n -> p ko n", p=P))
            vWe_sb = cst.tile([P, KL, VA], BF16, tag="vWe")
            nc.sync.dma_start(vWe_sb[:], vWe[:].rearrange("(ko p) n -> p ko n", p=P))
            bc_sb = cst.tile([P, 8 * KD], F32, tag="bcols")
            nc.sync.dma_start(bc_sb[:], bcols[:])
            vber_sb = cst.tile([1, VA], BF16, tag="vber")
            nc.sync.dma_start(vber_sb[:], vber[:])
            vld_sb = cst.tile([P, BL * MK], F32, tag="vld")
            nc.sync.dma_start(vld_sb[:], vld[:])
            onesr_sb = cst.tile([1, P], F32, tag="onesr")
            nc.sync.dma_start(onesr_sb[:], onesr[:])
            onesrb_sb = cst.tile([1, P], BF16, tag="onesrb")
            nc.sync.dma_start(onesrb_sb[:], onesrb[:])
            onescb_sb = cst.tile([P, 1], BF16, tag="onescb")
            nc.sync.dma_start(onescb_sb[:], onescb[:])
            onescf_sb = cst.tile([P, 1], F32, tag="onescf")
            nc.sync.dma_start(onescf_sb[:], onescf[:])
            id_sb = cst.tile([P, P], F32, tag="ident")
            nc.sync.dma_start(id_sb[:], ident[:])
            # batch-0 inputs next, so the first projections start early;
            # late-used weights (outW/d1W) load after.
            gnT0_sb = gnp.tile([P, KD, NQ], BF16, tag="gnT")
            nc.sync.dma_start(gnT0_sb[:], gnT[0].rearrange("(ko p) t -> p ko t", p=P))
            condT0_sb = big.tile([P, KL, NK], BF16, tag="big", name="condT0")
            nc.sync.dma_start(condT0_sb[:], condT[0].rearrange("(ko p) t -> p ko t", p=P))
            gnTf0_sb = zzp.tile([P, KD, NQ], F32, tag="zz")
            nc.sync.dma_start(gnTf0_sb[:], gnTf[0].rearrange("(ko p) t -> p ko t", p=P))
            outW_sb = cst.tile([P, KD, D], BF16, tag="outW")
            nc.sync.dma_start(outW_sb[:], outW[:].rearrange("(ko p) n -> p ko n", p=P))
            d1W_sb = cst.tile([P, KD, D], BF16, tag="d1W")
            nc.sync.dma_start(d1W_sb[:], d1W[:].rearrange("(ko p) n -> p ko n", p=P))

            qbe_c = bc_sb[:, 0 * KD:1 * KD]
            kbe_c = bc_sb[:, 1 * KD:2 * KD]
            outb_c = bc_sb[:, 2 * KD:3 * KD]
            d1b_c = bc_sb[:, 3 * KD:4 * KD]
            ln1g_c = bc_sb[:, 4 * KD:5 * KD]
            ln1b_c = bc_sb[:, 5 * KD:6 * KD]
            ln2g_c = bc_sb[:, 6 * KD:7 * KD]
            ln2b_c = bc_sb[:, 7 * KD:8 * KD]

            def layer_norm(x_sb, g_c, b_c, out_sb):
                """Feature-major LN over partitions (768 feats = 6 chunks).
                Stats fp32; dtypes follow x_sb."""
                fp = x_sb.dtype == F32
                ones = onescf_sb if fp else onescb_sb
                sum1 = spp.tile([P, NK], F32, tag="sp", name="sum1")
                sq_sb = big.tile([P, KD, NQ], BF16, tag="big")
                sum2 = spp.tile([P, NK], F32, tag="sp", name="sum2")
                for kc in range(KD):
                    nc.tensor.matmul(sum1[0:1, :NQ], ones[:], x_sb[:, kc, :],
                                     start=(kc == 0), stop=(kc == KD - 1))
                    nc.scalar.activation(sq_sb[:, kc, :], x_sb[:, kc, :], AF.Square)
                    nc.tensor.matmul(sum2[0:1, :NQ], onescb_sb[:], sq_sb[:, kc, :],
                                     start=(kc == 0), stop=(kc == KD - 1))
                m_sb = sml.tile([1, NQ], F32, tag="m")
                nc.vector.tensor_scalar_mul(m_sb[:], sum1[0:1, :NQ], 1.0 / D)
                e2_sb = sml.tile([1, NQ], F32, tag="e2")
                nc.vector.tensor_scalar(e2_sb[:], sum2[0:1, :NQ], 1.0 / D, EPS,
                                        ALU.mult, ALU.add)
                msq_sb = sml.tile([1, NQ], F32, tag="msq_dtmp", name="msq_sb")
                nc.vector.tensor_tensor(msq_sb[:], m_sb[:], m_sb[:], ALU.mult)
                nc.vector.tensor_tensor(e2_sb[:], e2_sb[:], msq_sb[:], ALU.subtract)
                sd_sb = sml.tile([1, NQ], F32, tag="sd_rtmp", name="sd_sb")
                nc.scalar.activation(sd_sb[:], e2_sb[:], AF.Sqrt)
                rs_sb = sml.tile([1, NQ], F32, tag="rs_lnt", name="rs_sb")
                nc.vector.reciprocal(rs_sb[:], sd_sb[:])
                m_bc = spp.tile([P, NK], F32, tag="sp", name="m_bc")
                nc.tensor.matmul(m_bc[:, :NQ], onesr_sb[:], m_sb[:], start=True, stop=True)
                rs_bc = spp.tile([P, NK], F32, tag="sp", name="rs_bc")
                nc.tensor.matmul(rs_bc[:, :NQ], onesr_sb[:], rs_sb[:], start=True, stop=True)
                for kc in range(KD):
                    t_sb = sml.tile([P, NQ], F32, tag="rs_lnt", name="t_sb")
                    nc.vector.tensor_tensor(t_sb[:], x_sb[:, kc, :], m_bc[:, :NQ], ALU.subtract)
                    nc.vector.tensor_tensor(t_sb[:], t_sb[:], rs_bc[:, :NQ], ALU.mult)
                    nc.vector.tensor_scalar(out_sb[:, kc, :], t_sb[:],
                                            g_c[:, kc:kc + 1], b_c[:, kc:kc + 1],
                                            ALU.mult, ALU.add)

            for b in range(BL):
                # ---- input DMAs ----------------------------------------
                if b == 0:
                    gnT_sb, gnTf_sb, condT_sb = gnT0_sb, gnTf0_sb, condT0_sb
                else:
                    gnT_sb = gnp.tile([P, KD, NQ], BF16, tag="gnT")
                    nc.sync.dma_start(gnT_sb[:], gnT[b].rearrange("(ko p) t -> p ko t", p=P))
                    gnTf_sb = zzp.tile([P, KD, NQ], F32, tag="zz")
                    nc.sync.dma_start(gnTf_sb[:], gnTf[b].rearrange("(ko p) t -> p ko t", p=P))
                    condT_sb = big.tile([P, KL, NK], BF16, tag="big")
                    nc.sync.dma_start(condT_sb[:], condT[b].rearrange("(ko p) t -> p ko t", p=P))

                # ---- projections (feature-major, bf16 outputs) ----------
                qhT_sb = qhp.tile([P, KD, NQ], BF16, tag="qh")
                for m in range(KD):
                    ps = mmp.tile([P, 512], F32, tag="mm")
                    for kc in range(KD):
                        nc.tensor.matmul(ps[:, :NQ], qWe_sb[:, kc, m * P:(m + 1) * P],
                                         gnT_sb[:, kc, :], start=(kc == 0), stop=(kc == KD - 1))
                    nc.vector.tensor_scalar_add(qhT_sb[:, m, :], ps[:, :NQ], qbe_c[:, m:m + 1])

                khT_sb = khp.tile([P, KD, NK], BF16, tag="kh")
                for m in range(KD):
                    ps = spp.tile([P, NK], F32, tag="sp")
                    for n in range(2):
                        for kc in range(KL):
                            nc.tensor.matmul(ps[:, n * 512:(n + 1) * 512],
                                             kWe_sb[:, kc, m * P:(m + 1) * P],
                                             condT_sb[:, kc, n * 512:(n + 1) * 512],
                                             start=(kc == 0), stop=(kc == KL - 1))
                    nc.vector.tensor_scalar_add(khT_sb[:, m, :], ps[:], kbe_c[:, m:m + 1])

                vh_sb = vhp.tile([P, MK, VA], BF16, tag="vh")
                for mk in range(MK):
                    ps = spp.tile([P, NK], F32, tag="sp")
                    for (n0, nw) in ((0, 512), (512, VA - 512)):
                        for kc in range(KL):
                            nc.tensor.matmul(ps[:, n0:n0 + nw],
                                             condT_sb[:, kc, mk * P:(mk + 1) * P],
                                             vWe_sb[:, kc, n0:n0 + nw],
                                             start=(kc == 0), stop=False)
                        nc.tensor.matmul(ps[:, n0:n0 + nw], onesrb_sb[:],
                                         vber_sb[:, n0:n0 + nw], start=False, stop=True)
                    # bias included; now zero masked key rows (incl ones-col)
                    nc.scalar.activation(vh_sb[:, mk, :], ps[:, :VA], AF.Copy,
                                         scale=vld_sb[:, b * MK + mk: b * MK + mk + 1])

                # ---- attention (head pairs, feature-major) --------------
                ctxT_sb = ctp.tile([P, KD, NQ], BF16, tag="ctx")
                def s_block(hp, pT):
                    th = hp
                    for g2 in range(MK // 2):
                        s_ps = [spp.tile([P, NK], F32, tag="sp", name=f"s_ps{e}") for e in range(2)]
                        for half in range(2):
                            mk = 2 * g2 + half
                            for e in range(2):   # even/odd head interleaved
                                off = DH * e
                                nc.tensor.matmul(s_ps[e][:, half * NQ:(half + 1) * NQ],
                                                 khT_sb[off:off + DH, th, mk * P:(mk + 1) * P],
                                                 qhT_sb[off:off + DH, th, :],
                                                 start=True, stop=True)
                        for e in range(2):
                            nc.scalar.activation(pT[e][:, 2 * g2:2 * g2 + 2, :], s_ps[e][:], AF.Exp)

                for hp in range(H // 2):
                    th = hp
                    pT = [big.tile([P, MK, NQ], BF16, tag="big", name=f"pT{e}") for e in range(2)]
                    s_block(hp, pT)
                    for e in range(2):
                        h = 2 * hp + e
                        off = DH * e
                        c_ps = mmp.tile([P, 512], F32, tag="mm")
                        for kc in range(MK):
                            nc.tensor.matmul(c_ps[0:DH + 1, :NQ],
                                             vh_sb[:, kc, h * (DH + 1):(h + 1) * (DH + 1)],
                                             pT[e][:, kc, :],
                                             start=(kc == 0), stop=(kc == MK - 1))
                        # softmax denominator -> reciprocal -> PE broadcast
                        dtmp = sml.tile([1, NQ], F32, tag="msq_dtmp", name="dtmp")
                        nc.scalar.copy(dtmp[:], c_ps[DH:DH + 1, :NQ])
                        rtmp = sml.tile([1, NQ], F32, tag="sd_rtmp", name="rtmp")
                        nc.vector.reciprocal(rtmp[:], dtmp[:])
                        r_ps = mmp.tile([P, 512], F32, tag="mm")
                        nc.tensor.matmul(r_ps[0:DH, :NQ], onesr_sb[0:1, 0:DH], rtmp[:],
                                         start=True, stop=True)
                        nc.vector.tensor_copy(ctxT_sb[off:off + DH, th, :], c_ps[0:DH, :NQ])
                        nc.vector.tensor_tensor(ctxT_sb[off:off + DH, th, :],
                                                ctxT_sb[off:off + DH, th, :],
                                                r_ps[0:DH, :NQ], ALU.mult)

                # ---- out-proj + residual + LN1 --------------------------
                xres_sb = xrp.tile([P, KD, NQ], F32, tag="xres")
                for m in range(KD):
                    ps = mmp.tile([P, 512], F32, tag="mm")
                    for kc in range(KD):
                        nc.tensor.matmul(ps[:, :NQ], outW_sb[:, kc, m * P:(m + 1) * P],
                                         ctxT_sb[:, kc, :], start=(kc == 0), stop=(kc == KD - 1))
                    t_sb = sml.tile([P, NQ], F32, tag="rs_lnt", name="t_sb")
                    nc.vector.tensor_scalar_add(t_sb[:], ps[:, :NQ], outb_c[:, m:m + 1])
                    nc.vector.tensor_tensor(xres_sb[:, m, :], t_sb[:],
                                            gnTf_sb[:, m, :], ALU.add)
                layer_norm(xres_sb, ln1g_c, ln1b_c, xres_sb)
                xbf_sb = qhp.tile([P, KD, NQ], BF16, tag="qh", name="xbf_sb")
                for m in range(KD):
                    nc.vector.tensor_copy(xbf_sb[:, m, :], xres_sb[:, m, :])

                # ---- FFN: y = leaky_relu(x @ d1W + d1b) + x, then LN2 ----
                y_sb = yyp.tile([P, KD, NQ], F32, tag="yy")
                for m in range(KD):
                    ps = mmp.tile([P, 512], F32, tag="mm")
                    for kc in range(KD):
                        nc.tensor.matmul(ps[:, :NQ], d1W_sb[:, kc, m * P:(m + 1) * P],
                                         xbf_sb[:, kc, :], start=(kc == 0), stop=(kc == KD - 1))
                    t_sb = sml.tile([P, NQ], F32, tag="rs_lnt", name="t_sb")
                    nc.scalar.activation(t_sb[:], ps[:, :NQ], AF.Lrelu,
                                         bias=d1b_c[:, m:m + 1], alpha=0.01)
                    nc.vector.tensor_tensor(y_sb[:, m, :], t_sb[:],
                                            xres_sb[:, m, :], ALU.add)
                z_sb = zzp.tile([P, KD, NQ], F32, tag="zz")
                layer_norm(y_sb, ln2g_c, ln2b_c, z_sb)

                # ---- transpose back to [tok, feat] and store ------------
                out_sb = otp.tile([P, MQ, D], F32, tag="outp")
                for t in range(MQ):
                    for m in range(KD):
                        tr_ps = mmp.tile([P, 512], F32, tag="mm")
                        nc.tensor.transpose(tr_ps[:, :P], z_sb[:, m, t * P:(t + 1) * P], id_sb[:])
                        nc.vector.tensor_copy(out_sb[:, t, m * P:(m + 1) * P], tr_ps[:, :P])
                    nc.sync.dma_start(out[b, t * P:(t + 1) * P, :], out_sb[:, t, :])

    nc.compile()
    return nc


def kernel(**inputs):
    gn = np.asarray(inputs["graph_nodes"], dtype=np.float32)
    cond = np.asarray(inputs["conditioning_vector"], dtype=np.float32)
    mask = np.asarray(inputs["conditioning_attention_mask"])
    g = lambda k: np.asarray(inputs[k], dtype=np.float32)

    qW, qb = g("qW"), g("qb")
    kW, kb = g("kW"), g("kb")
    vW, vb = g("vW"), g("vb")
    in_qW, in_qb = g("in_qW"), g("in_qb")
    in_kW, in_kb = g("in_kW"), g("in_kb")
    in_vW, in_vb = g("in_vW"), g("in_vb")
    outW, outb = g("outW"), g("outb")
    ln1g, ln1b = g("ln1g"), g("ln1b")
    d1W, d1b = g("d1W"), g("d1b")
    ln2g, ln2b = g("ln2g"), g("ln2b")

    scale = 1.0 / np.sqrt(np.float32(DH))
    qWe = (qW @ in_qW) * scale
    qbe = (qb @ in_qW + in_qb) * scale
    kWe = kW @ in_kW
    kbe = kb @ in_kW + in_kb
    vWe = vW @ in_vW
    vbe = vb @ in_vW + in_vb

    # vWe augmented with a zero column per head; bias row carries vbe + ones
    vWe_aug = np.zeros((L, VA), np.float32)
    vbe_aug = np.zeros((VA,), np.float32)
    for h in range(H):
        vWe_aug[:, h * (DH + 1):h * (DH + 1) + DH] = vWe[:, h * DH:(h + 1) * DH]
        vbe_aug[h * (DH + 1):h * (DH + 1) + DH] = vbe[h * DH:(h + 1) * DH]
        vbe_aug[h * (DH + 1) + DH] = 1.0

    col = lambda v: np.ascontiguousarray(v.reshape(KD, P).T, dtype=np.float32)  # [P, KD]
    bcols = np.concatenate(
        [col(qbe), col(kbe), col(outb), col(d1b),
         col(ln1g), col(ln1b), col(ln2g), col(ln2b)], axis=1)

    valid01 = np.where(mask, 0.0, 1.0).astype(np.float32)  # [B, NK]

    key = "nc"
    if key not in _NC_CACHE:
        _NC_CACHE[key] = _build_nc()
    nc = _NC_CACHE[key]

    bf = lambda a: np.ascontiguousarray(a.astype(NPBF))
    shared = {
        "qWe": bf(qWe), "kWe": bf(kWe), "vWe": bf(vWe_aug),
        "outW": bf(outW), "d1W": bf(d1W),
        "bcols": np.ascontiguousarray(bcols),
        "vber": bf(vbe_aug[None, :]),
        "onesr": np.ones((1, P), np.float32),
        "onesrb": np.ones((1, P), NPBF),
        "onescb": np.ones((P, 1), NPBF),
        "onescf": np.ones((P, 1), np.float32),
        "ident": np.eye(P, dtype=np.float32),
    }
    in_maps = []
    for c in range(NCORES):
        bs = slice(c * BL, (c + 1) * BL)
        vp = np.zeros((P, BL * MK), np.float32)
        for i, bb in enumerate(range(c * BL, (c + 1) * BL)):
            vp[:, i * MK:(i + 1) * MK] = valid01[bb].reshape(MK, P).T
        in_maps.append({
            **shared,
            "gnT": bf(gn[bs].transpose(0, 2, 1)),
            "gnTf": np.ascontiguousarray(gn[bs].transpose(0, 2, 1)),
            "condT": bf(cond[bs].transpose(0, 2, 1)),
            "vld": vp,
        })

    res = run_bass_kernel_spmd(nc, in_maps, list(range(NCORES)))
    return np.concatenate([res.results[c]["out"] for c in range(NCORES)], axis=0)



# revision 9
# speedup vs baseline: 1.3103x; 1.3103x over previous
"""Trainium2 Bass kernel for nn_CrossAttentionGraphBlock (v2).

Hardcoded for B=16, NQ=512, NK=1024, D=768, L=512, H=12, DH=64; 8 cores,
2 batches per core, data-parallel, no collectives.

Design notes (cost-model driven):
 - All linear projections (q/k/v fused with in-proj, out-proj, ctx@V) run in
   fp8e4m3 with DoubleRow perf mode: 0.5 cycles/row vs 1 for bf16, with
   host-side weight pre-scales; descales fold into PSUM->SBUF copies.
 - Attention context is accumulated token-major (out [q, dh] per head): the
   softmax denominator becomes a per-partition scalar -> one reciprocal + a
   per-partition tensor_scalar multiply. Denominators come from 1-column
   DoubleRow matmuls with the (x0.25) key-validity mask as moving operand;
   padded keys are zeroed in vh via a mask scale in its PSUM->SBUF copy.
 - With ln1g=1, ln1b=0, d1b=0 (the module's init), leaky_relu's positive
   homogeneity makes LN1's 1/std cancel inside LN2, so LN1 is just a mean
   subtraction. A general path handles arbitrary affines/biases.
 - Two batches are software-pipelined (A0 A1 S0 [ctx0|S1] C0 ctx1 C1) to keep
   the PE continuously busy (p-state) and overlap the scalar-engine exp wall.
 - Elementwise work is spread across DVE / Act / GPSIMD; Act is reserved
   mostly for the 96 softmax exp instructions.
"""

import numpy as np
import ml_dtypes

import concourse.bass as bass
import concourse.mybir as mybir
import concourse.tile as tile
from concourse import bacc
from concourse.bass_utils import run_bass_kernel_spmd

P = 128
B, NQ, NK, D, L, H = 16, 512, 1024, 768, 512, 12
DH = D // H          # 64
NCORES = 8
BL = B // NCORES     # 2 batches per core
EPS = 1e-5
F32 = mybir.dt.float32
BF16 = mybir.dt.bfloat16
F8 = mybir.dt.float8e4
NPBF = ml_dtypes.bfloat16
NPF8 = ml_dtypes.float8_e4m3fn
AF = mybir.ActivationFunctionType
ALU = mybir.AluOpType
PM = mybir.MatmulPerfMode

KD = D // P          # 6
KL = L // P          # 4
MQ = NQ // P         # 4
MK = NK // P         # 8
HP = H // 2          # 6

SQ = 512.0           # qWe scale (qWe includes 1/sqrt(DH))
SK = 64.0            # kWe scale
SV = 16.0            # vWe scale
SO = 32.0            # outW scale

_NC_CACHE = {}


def _build_nc(trivial_affine):
    nc = bacc.Bacc()

    gn8 = nc.declare_dram_parameter("gn8", [BL, D, NQ], F8, isOutput=False)
    gnT = nc.declare_dram_parameter("gnT", [BL, D, NQ], BF16, isOutput=False)
    cond8 = nc.declare_dram_parameter("cond8", [BL, L, NK], F8, isOutput=False)
    qWe8 = nc.declare_dram_parameter("qWe8", [D, D], F8, isOutput=False)
    kWe8 = nc.declare_dram_parameter("kWe8", [L, D], F8, isOutput=False)
    vWe8 = nc.declare_dram_parameter("vWe8", [L, D], F8, isOutput=False)
    outW8 = nc.declare_dram_parameter("outW8", [D, D], F8, isOutput=False)
    d1W = nc.declare_dram_parameter("d1W", [D, D], BF16, isOutput=False)
    vld = nc.declare_dram_parameter("vld", [P, BL * MK], F32, isOutput=False)
    vld8 = nc.declare_dram_parameter("vld8", [P, BL * MK, 1], F8, isOutput=False)
    bcols = nc.declare_dram_parameter("bcols", [P, 8 * KD], F32, isOutput=False)
    vber = nc.declare_dram_parameter("vber", [1, D], BF16, isOutput=False)
    onesr = nc.declare_dram_parameter("onesr", [1, P], BF16, isOutput=False)
    onesc = nc.declare_dram_parameter("onesc", [P, 1], BF16, isOutput=False)
    out = nc.declare_dram_parameter("out", [BL, NQ, D], F32, isOutput=True)

    DESC_Q = 1.0 / SQ
    DESC_K = 1.0 / SK
    DESC_O = 1.0 / (SV * 4 * SO)   # ctx8 = 4*SV*ctx_norm (vld8 carries 0.25)

    from contextlib import ExitStack
    with tile.TileContext(nc) as tc:
        with ExitStack() as _es:
            pool = lambda *a, **k: _es.enter_context(tc.tile_pool(*a, **k))
            cst = pool(name="cst", bufs=1)
            gn8p = pool(name="gn8p", bufs=2)
            gnTp = pool(name="gnTp", bufs=2)
            cond8p = pool(name="cond8p", bufs=2)
            qhp = pool(name="qhp", bufs=2)
            khp = pool(name="khp", bufs=2)
            vhp = pool(name="vhp", bufs=2)
            ptp = pool(name="ptp", bufs=10)
            ctokp = pool(name="ctokp", bufs=1)
            ctxTbp = pool(name="ctxTbp", bufs=1)
            ctxTp = pool(name="ctxTp", bufs=2)
            zTp = pool(name="zTp", bufs=1)
            xp = pool(name="xp", bufs=1)
            yp = pool(name="yp", bufs=1)
            zp = pool(name="zp", bufs=1)
            tbp = pool(name="tb", bufs=3)
            yqp = pool(name="yq", bufs=2)
            outp = pool(name="outp", bufs=2)
            rwp = pool(name="rw", bufs=4)
            rcp = pool(name="rc", bufs=4)
            bigp = pool(name="ps1024", bufs=2, space="PSUM")
            mmp = pool(name="mm512", bufs=3, space="PSUM")
            # ---------------- DMAs, earliest-needed first -----------------
            cond8_sb = [None] * BL
            gn8_sb = [None] * BL
            gnT_sb = [None] * BL
            cond8_sb[0] = cond8p.tile([P, KL, NK], F8, tag="cond8", name="cond8_0")
            nc.sync.dma_start(cond8_sb[0][:], cond8[0].rearrange("(ko p) t -> p ko t", p=P))
            kWe8_sb = cst.tile([P, KL, D], F8, tag="kWe8")
            nc.sync.dma_start(kWe8_sb[:], kWe8[:].rearrange("(ko p) n -> p ko n", p=P))
            gn8_sb[0] = gn8p.tile([P, KD, NQ], F8, tag="gn8", name="gn8_0")
            nc.sync.dma_start(gn8_sb[0][:], gn8[0].rearrange("(ko p) t -> p ko t", p=P))
            qWe8_sb = cst.tile([P, KD, D], F8, tag="qWe8")
            nc.sync.dma_start(qWe8_sb[:], qWe8[:].rearrange("(ko p) n -> p ko n", p=P))
            vWe8_sb = cst.tile([P, KL, D], F8, tag="vWe8")
            nc.sync.dma_start(vWe8_sb[:], vWe8[:].rearrange("(ko p) n -> p ko n", p=P))
            vld_sb = cst.tile([P, BL * MK], F32, tag="vld")
            nc.sync.dma_start(vld_sb[:], vld[:])
            vld8_sb = cst.tile([P, BL * MK, 1], F8, tag="vld8")
            nc.sync.dma_start(vld8_sb[:], vld8[:])
            onesr_sb = cst.tile([1, P], BF16, tag="onesr")
            nc.sync.dma_start(onesr_sb[:], onesr[:])
            onesc_sb = cst.tile([P, 1], BF16, tag="onesc")
            nc.sync.dma_start(onesc_sb[:], onesc[:])
            bc_sb = cst.tile([P, 8 * KD], F32, tag="bcols")
            nc.sync.dma_start(bc_sb[:], bcols[:])
            vber_sb = cst.tile([1, D], BF16, tag="vber")
            nc.sync.dma_start(vber_sb[:], vber[:])
            cond8_sb[1] = cond8p.tile([P, KL, NK], F8, tag="cond8", name="cond8_1")
            nc.sync.dma_start(cond8_sb[1][:], cond8[1].rearrange("(ko p) t -> p ko t", p=P))
            gn8_sb[1] = gn8p.tile([P, KD, NQ], F8, tag="gn8", name="gn8_1")
            nc.sync.dma_start(gn8_sb[1][:], gn8[1].rearrange("(ko p) t -> p ko t", p=P))
            gnT_sb[0] = gnTp.tile([P, KD, NQ], BF16, tag="gnT", name="gnT_0")
            nc.sync.dma_start(gnT_sb[0][:], gnT[0].rearrange("(ko p) t -> p ko t", p=P))
            outW8_sb = cst.tile([P, KD, D], F8, tag="outW8")
            nc.sync.dma_start(outW8_sb[:], outW8[:].rearrange("(ko p) n -> p ko n", p=P))
            d1W_sb = cst.tile([P, KD, D], BF16, tag="d1W")
            nc.sync.dma_start(d1W_sb[:], d1W[:].rearrange("(ko p) n -> p ko n", p=P))
            gnT_sb[1] = gnTp.tile([P, KD, NQ], BF16, tag="gnT", name="gnT_1")
            nc.sync.dma_start(gnT_sb[1][:], gnT[1].rearrange("(ko p) t -> p ko t", p=P))

            _es.enter_context(nc.allow_low_precision(
                reason="bf16 LN stats are within tolerance; residual dominates"))
            qbe_c = bc_sb[:, 0 * KD:1 * KD]
            kbe_c = bc_sb[:, 1 * KD:2 * KD]
            outb_c = bc_sb[:, 2 * KD:3 * KD]
            d1b_c = bc_sb[:, 3 * KD:4 * KD]
            ln1g_c = bc_sb[:, 4 * KD:5 * KD]
            ln1b_c = bc_sb[:, 5 * KD:6 * KD]
            ln2g_c = bc_sb[:, 6 * KD:7 * KD]
            ln2b_c = bc_sb[:, 7 * KD:8 * KD]

            qhT = [None] * BL
            khT = [None] * BL
            vh8 = [None] * BL
            ctok8 = [None] * BL
            ctxTb = [None] * BL
            ctxT8 = [None] * BL
            pt8 = [[None] * HP for _ in range(BL)]

            # ---------------- phase A: projections ------------------------
            def phase_A(b):
                khT[b] = khp.tile([P, KD, NK], BF16, tag="khT", name="khT")
                for m in range(KD):
                    ks = bigp.tile([P, NK], F32, tag="ps1024", name="ks")
                    for n in range(2):
                        for i in range(KL // 2):
                            nc.tensor.matmul(ks[:, n * 512:(n + 1) * 512],
                                             kWe8_sb[:, 2 * i:2 * i + 2, m * P:(m + 1) * P],
                                             cond8_sb[b][:, 2 * i:2 * i + 2, n * 512:(n + 1) * 512],
                                             start=(i == 0), stop=(i == KL // 2 - 1),
                                             perf_mode=PM.DoubleRow)
                    if trivial_affine:
                        nc.vector.tensor_scalar_mul(khT[b][:, m, :], ks[:, :NK], DESC_K)
                    else:
                        nc.vector.tensor_scalar(khT[b][:, m, :], ks[:, :NK], DESC_K,
                                                kbe_c[:, m:m + 1], ALU.mult, ALU.add)
                qhT[b] = qhp.tile([P, KD, NQ], BF16, tag="qhT", name="qhT")
                for m in range(KD):
                    qs = mmp.tile([P, 512], F32, tag="mm512", name="qs")
                    for i in range(KD // 2):
                        nc.tensor.matmul(qs[:, :NQ], qWe8_sb[:, 2 * i:2 * i + 2, m * P:(m + 1) * P],
                                         gn8_sb[b][:, 2 * i:2 * i + 2, :],
                                         start=(i == 0), stop=(i == KD // 2 - 1),
                                         perf_mode=PM.DoubleRow)
                    if trivial_affine:
                        nc.vector.tensor_scalar_mul(qhT[b][:, m, :], qs[:, :NQ], DESC_Q)
                    else:
                        nc.vector.tensor_scalar(qhT[b][:, m, :], qs[:, :NQ], DESC_Q,
                                                qbe_c[:, m:m + 1], ALU.mult, ALU.add)
                vh8[b] = vhp.tile([P, MK, D], F8, tag="vh8", name="vh8")
                for mk in range(MK):
                    vs = bigp.tile([P, NK], F32, tag="ps1024", name="vs")
                    for (n0, nw) in ((0, 512), (512, 256)):
                        for i in range(KL // 2):
                            nc.tensor.matmul(vs[:, n0:n0 + nw],
                                             cond8_sb[b][:, 2 * i:2 * i + 2, mk * P:(mk + 1) * P],
                                             vWe8_sb[:, 2 * i:2 * i + 2, n0:n0 + nw],
                                             start=(i == 0), stop=(i == KL // 2 - 1 and trivial_affine),
                                             perf_mode=PM.DoubleRow)
                    if not trivial_affine:
                        for (n0, nw) in ((0, 512), (512, 256)):
                            nc.tensor.matmul(vs[:, n0:n0 + nw], onesr_sb[0:1, 0:P],
                                             vber_sb[:, n0:n0 + nw], start=False, stop=True,
                                             skip_group_check=True)
                    nc.vector.tensor_scalar_mul(vh8[b][:, mk, :], vs[:, :D],
                                                vld_sb[:, b * MK + mk:b * MK + mk + 1])

            # ---------------- phase B: attention --------------------------
            def s_exp(b, hp):
                pt8[b][hp] = [ptp.tile([P, MK, NQ], F8, tag="pt8", name=f"pt{e}")
                              for e in range(2)]
                for g2 in range(MK // 2):
                    sps = [bigp.tile([P, NK], F32, tag="ps1024", name=f"s{e}")
                           for e in range(2)]
                    for half in range(2):
                        mk = 2 * g2 + half
                        for e in range(2):
                            off = DH * e
                            nc.tensor.matmul(sps[e][:, half * NQ:(half + 1) * NQ],
                                             khT[b][off:off + DH, hp, mk * P:(mk + 1) * P],
                                             qhT[b][off:off + DH, hp, :],
                                             start=True, stop=True)
                    for e in range(2):
                        nc.scalar.activation(pt8[b][hp][e][:, 2 * g2:2 * g2 + 2, :],
                                             sps[e][:], AF.Exp)

            def ctx_pair(b, hp):
                if ctok8[b] is None:
                    ctok8[b] = ctokp.tile([P, MQ, H, DH], BF16, tag="ctok8", name="ctok8")
                    ctxTb[b] = ctxTbp.tile([P, KD, NQ], BF16, tag="ctxTb", name="ctxTb")
                dn = mmp.tile([P, 512], F32, tag="mm512", name="dn")
                cps = [None, None]
                for e in range(2):
                    h = 2 * hp + e
                    pt = pt8[b][hp][e]
                    cps[e] = mmp.tile([P, 512], F32, tag="mm512", name=f"cps{e}")
                    for mq in range(MQ):
                        for i in range(MK // 2):
                            nc.tensor.matmul(cps[e][:, mq * DH:(mq + 1) * DH],
                                             pt[:, 2 * i:2 * i + 2, mq * P:(mq + 1) * P],
                                             vh8[b][:, 2 * i:2 * i + 2, h * DH:(h + 1) * DH],
                                             start=(i == 0), stop=(i == MK // 2 - 1),
                                             perf_mode=PM.DoubleRow)
                        for i in range(MK // 2):
                            nc.tensor.matmul(dn[:, mq * 2 + e:mq * 2 + e + 1],
                                             pt[:, 2 * i:2 * i + 2, mq * P:(mq + 1) * P],
                                             vld8_sb[:, b * MK + 2 * i:b * MK + 2 * i + 2, :],
                                             start=(i == 0), stop=(i == MK // 2 - 1),
                                             perf_mode=PM.DoubleRow)
                rc = rcp.tile([P, 2 * MQ], F32, tag="rc")
                nc.vector.reciprocal(rc[:], dn[:, 0:2 * MQ])
                for mq in range(MQ):
                    for e in range(2):
                        h = 2 * hp + e
                        nc.vector.tensor_scalar_mul(ctok8[b][:, mq, h, :],
                                                    cps[e][:, mq * DH:(mq + 1) * DH],
                                                    rc[:, mq * 2 + e:mq * 2 + e + 1])
                for mq in range(MQ):
                    nc.sync.dma_start_transpose(ctxTb[b][:, hp, mq * P:(mq + 1) * P],
                                                ctok8[b][:, mq, 2 * hp:2 * hp + 2, :])

            # ---------------- phase C -------------------------------------
            def phase_C(b):
                ctxT8[b] = ctxTp.tile([P, KD, NQ], F8, tag="ctxT8", name="ctxT8")
                for m in range(KD):
                    nc.gpsimd.tensor_copy(ctxT8[b][:, m, :], ctxTb[b][:, m, :])
                x = xp.tile([P, KD, NQ], BF16, tag="x")
                for m in range(KD):
                    om = mmp.tile([P, 512], F32, tag="mm512", name="om")
                    for i in range(KD // 2):
                        nc.tensor.matmul(om[:, :NQ], outW8_sb[:, 2 * i:2 * i + 2, m * P:(m + 1) * P],
                                         ctxT8[b][:, 2 * i:2 * i + 2, :],
                                         start=(i == 0), stop=(i == KD // 2 - 1),
                                         perf_mode=PM.DoubleRow)
                    t0 = tbp.tile([P, NQ], BF16, tag="tb", name="t0")
                    if trivial_affine:
                        nc.vector.tensor_scalar_mul(t0[:], om[:, :NQ], DESC_O)
                    else:
                        nc.vector.tensor_scalar(t0[:], om[:, :NQ], DESC_O,
                                                outb_c[:, m:m + 1], ALU.mult, ALU.add)
                    nc.gpsimd.tensor_tensor(x[:, m, :], t0[:], gnT_sb[b][:, m, :], ALU.add)
                # LN1
                ms = mmp.tile([P, 512], F32, tag="mm512", name="ms")
                for m in range(KD):
                    nc.tensor.matmul(ms[0:1, :NQ], onesc_sb[:], x[:, m, :],
                                     start=(m == 0), stop=(m == KD - 1))
                m_sb = rwp.tile([1, NQ], BF16, tag="rw", name="m_sb")
                nc.vector.tensor_scalar_mul(m_sb[:], ms[0:1, :NQ], 1.0 / D)
                if not trivial_affine:
                    sqs = mmp.tile([P, 512], F32, tag="mm512", name="sqs")
                    for m in range(KD):
                        xq = yqp.tile([P, NQ], BF16, tag="yq", name="xq")
                        nc.vector.tensor_tensor(xq[:], x[:, m, :], x[:, m, :], ALU.mult)
                        nc.tensor.matmul(sqs[0:1, :NQ], onesc_sb[:], xq[:],
                                         start=(m == 0), stop=(m == KD - 1))
                    e2 = rwp.tile([1, NQ], F32, tag="rw", name="e2")
                    nc.vector.tensor_scalar(e2[:], sqs[0:1, :NQ], 1.0 / D, EPS, ALU.mult, ALU.add)
                    mf = rwp.tile([1, NQ], F32, tag="rw", name="mf")
                    nc.vector.tensor_copy(mf[:], m_sb[:])
                    nc.vector.tensor_tensor(mf[:], mf[:], mf[:], ALU.mult)
                    nc.vector.tensor_tensor(e2[:], e2[:], mf[:], ALU.subtract)
                    sd = rwp.tile([1, NQ], F32, tag="rw", name="sd")
                    nc.scalar.activation(sd[:], e2[:], AF.Sqrt)
                    rs = rwp.tile([1, NQ], BF16, tag="rw", name="rs")
                    nc.vector.reciprocal(rs[:], sd[:])
                mb = mmp.tile([P, 512], F32, tag="mm512", name="mb")
                nc.tensor.matmul(mb[:, :NQ], onesr_sb[:], m_sb[:], start=True, stop=True)
                u = x
                if trivial_affine:
                    mb_s = tbp.tile([P, NQ], F32, tag="tb", name="mb_s")
                    nc.vector.tensor_copy(mb_s[:], mb[:, :NQ])
                    for m in range(KD):
                        nc.gpsimd.tensor_tensor(u[:, m, :], x[:, m, :], mb_s[:], ALU.subtract)
                else:
                    rb = mmp.tile([P, 512], F32, tag="mm512", name="rb")
                    nc.tensor.matmul(rb[:, :NQ], onesr_sb[:], rs[:], start=True, stop=True)
                    for m in range(KD):
                        nc.vector.tensor_tensor(u[:, m, :], x[:, m, :], mb[:, :NQ], ALU.subtract)
                        nc.vector.tensor_tensor(u[:, m, :], u[:, m, :], rb[:, :NQ], ALU.mult)
                        nc.vector.tensor_scalar(u[:, m, :], u[:, m, :], ln1g_c[:, m:m + 1],
                                                ln1b_c[:, m:m + 1], ALU.mult, ALU.add)
                # FFN
                y = yp.tile([P, KD, NQ], BF16, tag="y")
                for m in range(KD):
                    fm = mmp.tile([P, 512], F32, tag="mm512", name="fm")
                    for kc in range(KD):
                        nc.tensor.matmul(fm[:, :NQ], d1W_sb[:, kc, m * P:(m + 1) * P],
                                         u[:, kc, :], start=(kc == 0), stop=(kc == KD - 1))
                    t1 = tbp.tile([P, NQ], BF16, tag="tb", name="t1")
                    nc.scalar.activation(t1[:], fm[:, :NQ], AF.Lrelu,
                                         bias=d1b_c[:, m:m + 1], alpha=0.01)
                    nc.gpsimd.tensor_tensor(y[:, m, :], t1[:], u[:, m, :], ALU.add)
                # LN2
                s2 = mmp.tile([P, 512], F32, tag="mm512", name="s2")
                sq2 = mmp.tile([P, 512], F32, tag="mm512", name="sq2")
                for m in range(KD):
                    nc.tensor.matmul(s2[0:1, :NQ], onesc_sb[:], y[:, m, :],
                                     start=(m == 0), stop=(m == KD - 1))
                    yq = yqp.tile([P, NQ], BF16, tag="yq", name="yq")
                    nc.gpsimd.tensor_tensor(yq[:], y[:, m, :], y[:, m, :], ALU.mult)
                    nc.tensor.matmul(sq2[0:1, :NQ], onesc_sb[:], yq[:],
                                     start=(m == 0), stop=(m == KD - 1))
                m2 = rwp.tile([1, NQ], BF16, tag="rw", name="m2")
                nc.vector.tensor_scalar_mul(m2[:], s2[0:1, :NQ], 1.0 / D)
                e2b = rwp.tile([1, NQ], F32, tag="rw", name="e2b")
                nc.vector.tensor_scalar(e2b[:], sq2[0:1, :NQ], 1.0 / D, EPS, ALU.mult, ALU.add)
                m2f = rwp.tile([1, NQ], F32, tag="rw", name="m2f")
                nc.vector.tensor_copy(m2f[:], m2[:])
                nc.vector.tensor_tensor(m2f[:], m2f[:], m2f[:], ALU.mult)
                nc.vector.tensor_tensor(e2b[:], e2b[:], m2f[:], ALU.subtract)
                sd2 = rwp.tile([1, NQ], F32, tag="rw", name="sd2")
                nc.scalar.activation(sd2[:], e2b[:], AF.Sqrt)
                rs2 = rwp.tile([1, NQ], BF16, tag="rw", name="rs2")
                nc.vector.reciprocal(rs2[:], sd2[:])
                m2b = mmp.tile([P, 512], F32, tag="mm512", name="m2b")
                nc.tensor.matmul(m2b[:, :NQ], onesr_sb[:], m2[:], start=True, stop=True)
                r2b = mmp.tile([P, 512], F32, tag="mm512", name="r2b")
                nc.tensor.matmul(r2b[:, :NQ], onesr_sb[:], rs2[:], start=True, stop=True)
                m2b_s = tbp.tile([P, NQ], F32, tag="tb", name="m2b_s")
                nc.vector.tensor_copy(m2b_s[:], m2b[:, :NQ])
                r2b_s = tbp.tile([P, NQ], F32, tag="tb", name="r2b_s")
                nc.vector.tensor_copy(r2b_s[:], r2b[:, :NQ])
                z = zp.tile([P, KD, NQ], BF16, tag="z")
                for m in range(KD):
                    zt = tbp.tile([P, NQ], BF16, tag="tb", name="zt")
                    nc.gpsimd.tensor_tensor(zt[:], y[:, m, :], m2b_s[:], ALU.subtract)
                    nc.gpsimd.tensor_tensor(z[:, m, :], zt[:], r2b_s[:], ALU.mult)
                    if not trivial_affine:
                        nc.gpsimd.tensor_scalar(z[:, m, :], z[:, m, :], ln2g_c[:, m:m + 1],
                                                ln2b_c[:, m:m + 1], ALU.mult, ALU.add)
                zT = zTp.tile([P, MQ, D], BF16, tag="zT", name="zT")
                for t in range(MQ):
                    for m in range(KD):
                        nc.sync.dma_start_transpose(zT[:, t, m * P:(m + 1) * P],
                                                    z[:, m, t * P:(t + 1) * P])
                for t in range(MQ):
                    o_sb = outp.tile([P, D], F32, tag="outp")
                    nc.gpsimd.tensor_copy(o_sb[:], zT[:, t, :])
                    nc.sync.dma_start(out[b, t * P:(t + 1) * P, :], o_sb[:])

            # ---------------- schedule ------------------------------------
            phase_A(0)
            phase_A(1)
            for hp in range(HP - 1):
                s_exp(0, hp)
            ctx_pair(0, 0)
            s_exp(0, HP - 1)
            for hp in range(HP - 1):
                ctx_pair(0, hp + 1)
                s_exp(1, hp)
            ctx_pair(1, 0)
            s_exp(1, HP - 1)
            phase_C(0)
            for hp in range(1, HP):
                ctx_pair(1, hp)
            phase_C(1)

    nc.compile()
    return nc


def kernel(**inputs):
    gn = np.asarray(inputs["graph_nodes"], dtype=np.float32)
    cond = np.asarray(inputs["conditioning_vector"], dtype=np.float32)
    mask = np.asarray(inputs["conditioning_attention_mask"])
    g = lambda k: np.asarray(inputs[k], dtype=np.float32)

    qW, qb = g("qW"), g("qb")
    kW, kb = g("kW"), g("kb")
    vW, vb = g("vW"), g("vb")
    in_qW, in_qb = g("in_qW"), g("in_qb")
    in_kW, in_kb = g("in_kW"), g("in_kb")
    in_vW, in_vb = g("in_vW"), g("in_vb")
    outW, outb = g("outW"), g("outb")
    ln1g, ln1b = g("ln1g"), g("ln1b")
    d1W, d1b = g("d1W"), g("d1b")
    ln2g, ln2b = g("ln2g"), g("ln2b")

    scale = 1.0 / np.sqrt(np.float32(DH))
    qWe = (qW @ in_qW) * scale
    qbe = (qb @ in_qW + in_qb) * scale
    kWe = kW @ in_kW
    kbe = kb @ in_kW + in_kb
    vWe = vW @ in_vW
    vbe = vb @ in_vW + in_vb

    trivial = bool(
        np.all(qbe == 0) and np.all(kbe == 0) and np.all(vbe == 0)
        and np.all(outb == 0) and np.all(d1b == 0)
        and np.all(ln1g == 1) and np.all(ln1b == 0)
        and np.all(ln2g == 1) and np.all(ln2b == 0))

    col = lambda v: np.ascontiguousarray(v.reshape(KD, P).T, dtype=np.float32)
    bcols = np.concatenate(
        [col(qbe), col(kbe), col(outb), col(d1b),
         col(ln1g), col(ln1b), col(ln2g), col(ln2b)], axis=1)

    valid01 = np.where(mask, 0.0, 1.0).astype(np.float32)  # [B, NK]

    key = ("nc", trivial)
    if key not in _NC_CACHE:
        _NC_CACHE[key] = _build_nc(trivial)
        _NC_CACHE["nc"] = _NC_CACHE[key]
    nc = _NC_CACHE[key]

    f8 = lambda a: np.ascontiguousarray(np.clip(a, -448, 448).astype(NPF8))
    bf = lambda a: np.ascontiguousarray(a.astype(NPBF))
    shared = {
        "qWe8": f8(qWe * SQ), "kWe8": f8(kWe * SK), "vWe8": f8(vWe * SV),
        "outW8": f8(outW * SO), "d1W": bf(d1W),
        "bcols": np.ascontiguousarray(bcols),
        "vber": bf((SV * vbe)[None, :]),
        "onesr": np.ones((1, P), NPBF),
        "onesc": np.ones((P, 1), NPBF),
    }
    in_maps = []
    for c in range(NCORES):
        bs = slice(c * BL, (c + 1) * BL)
        vp = np.zeros((P, BL * MK), np.float32)
        for i, bb in enumerate(range(c * BL, (c + 1) * BL)):
            vp[:, i * MK:(i + 1) * MK] = valid01[bb].reshape(MK, P).T
        in_maps.append({
            **shared,
            "gn8": f8(gn[bs].transpose(0, 2, 1)),
            "gnT": bf(gn[bs].transpose(0, 2, 1)),
            "cond8": f8(cond[bs].transpose(0, 2, 1)),
            "vld": vp,
            "vld8": f8((vp * 0.25)[:, :, None]),
        })

    res = run_bass_kernel_spmd(nc, in_maps, list(range(NCORES)))
    return np.concatenate([res.results[c]["out"] for c in range(NCORES)], axis=0)


# revision 10
# speedup vs baseline: 1.5731x; 1.2005x over previous
"""Trainium2 Bass kernel for nn_CrossAttentionGraphBlock (v2.1).

Hardcoded for B=16, NQ=512, NK=1024, D=768, L=512, H=12, DH=64; 8 cores,
2 batches per core, data-parallel, no collectives.

Design notes (cost-model driven):
 - All linear projections (q/k/v fused with in-proj, out-proj, ctx@V) run in
   fp8e4m3 with DoubleRow perf mode (0.5 cycles/row); host-side weight
   pre-scales keep fp8 mantissas busy, descales fold into the PSUM->SBUF
   copies that are needed anyway.
 - Attention context is token-major: softmax denominators land in a
   per-partition column (1-column DoubleRow matmuls against the x0.25 key
   mask), normalized by one reciprocal + per-partition tensor_scalar mult.
   Padded keys are zeroed in vh via a mask scale in its PSUM->SBUF copy.
 - With ln1g=1, ln1b=0, d1b=0 (module init), leaky_relu positive homogeneity
   cancels LN1's 1/std inside LN2, so LN1 is a mean subtraction only.
 - [tok,feat]<->[feat,tok] transposes ride the idle DMA engines via blocked
   dma_start_transpose (out[q,k,p] = in[p, k*128+q]), one call per 512 cols.
 - k/q projections are emitted per 128-feature chunk, immediately followed by
   that head pair's S matmuls + exp, so the scalar engine (the exp wall,
   ~1us x 96) starts ~4us in and stays saturated; batch 2 interleaves behind
   batch 1's softmax; out-proj/FFN/LN of batch 1 overlap batch 2's softmax.
 - Elementwise: Act = exp + lrelu + some copies; DVE = PSUM copies/normalize
   + LN; GPSIMD (SBUF-only by HW rule) = residual adds and final casts.
"""

import numpy as np
import ml_dtypes

import concourse.bass as bass
import concourse.mybir as mybir
import concourse.tile as tile
from concourse import bacc
from concourse.bass_utils import run_bass_kernel_spmd

P = 128
B, NQ, NK, D, L, H = 16, 512, 1024, 768, 512, 12
DH = D // H
NCORES = 8
BL = B // NCORES
EPS = 1e-5
F32 = mybir.dt.float32
BF16 = mybir.dt.bfloat16
F8 = mybir.dt.float8e4
NPBF = ml_dtypes.bfloat16
NPF8 = ml_dtypes.float8_e4m3fn
AF = mybir.ActivationFunctionType
ALU = mybir.AluOpType
PM = mybir.MatmulPerfMode

KD = D // P          # 6
KL = L // P          # 4
MQ = NQ // P         # 4
MK = NK // P         # 8
HP = H // 2          # 6

SQ = 512.0           # qWe scale (qWe includes 1/sqrt(DH))
SK = 64.0            # kWe scale
SV = 16.0            # vWe scale
SO = 32.0            # outW scale

_NC_CACHE = {}


def _build_nc(trivial_affine):
    nc = bacc.Bacc()

    gn8 = nc.declare_dram_parameter("gn8", [BL, D, NQ], F8, isOutput=False)
    gnT = nc.declare_dram_parameter("gnT", [BL, D, NQ], BF16, isOutput=False)
    cond8 = nc.declare_dram_parameter("cond8", [BL, L, NK], F8, isOutput=False)
    qWe8 = nc.declare_dram_parameter("qWe8", [D, D], F8, isOutput=False)
    kWe8 = nc.declare_dram_parameter("kWe8", [L, D], F8, isOutput=False)
    vWe8 = nc.declare_dram_parameter("vWe8", [L, D], F8, isOutput=False)
    outW8 = nc.declare_dram_parameter("outW8", [D, D], F8, isOutput=False)
    d1W = nc.declare_dram_parameter("d1W", [D, D], BF16, isOutput=False)
    vld = nc.declare_dram_parameter("vld", [P, BL * MK], F32, isOutput=False)
    vld8 = nc.declare_dram_parameter("vld8", [P, BL * MK, 1], F8, isOutput=False)
    bcols = nc.declare_dram_parameter("bcols", [P, 8 * KD], F32, isOutput=False)
    vber = nc.declare_dram_parameter("vber", [1, D], BF16, isOutput=False)
    onesr = nc.declare_dram_parameter("onesr", [1, P], BF16, isOutput=False)
    onesc = nc.declare_dram_parameter("onesc", [P, 1], BF16, isOutput=False)
    out = nc.declare_dram_parameter("out", [BL, NQ, D], F32, isOutput=True)

    DESC_Q = 1.0 / SQ
    DESC_K = 1.0 / SK
    DESC_O = 1.0 / (SV * 4 * SO)   # ctx8 = 4*SV*ctx_norm (vld8 carries 0.25)

    from contextlib import ExitStack
    with tile.TileContext(nc) as tc:
        with ExitStack() as _es:
            pool = lambda *a, **k: _es.enter_context(tc.tile_pool(*a, **k))
            cst = pool(name="cst", bufs=1)
            gn8p = pool(name="gn8p", bufs=2)
            gnTp = pool(name="gnTp", bufs=2)
            cond8p = pool(name="cond8p", bufs=2)
            qhp = pool(name="qhp", bufs=2)
            khp = pool(name="khp", bufs=2)
            vhp = pool(name="vhp", bufs=2)
            ptp = pool(name="ptp", bufs=10)
            ctokp = pool(name="ctokp", bufs=1)
            ctxTbp = pool(name="ctxTbp", bufs=1)
            ctxTp = pool(name="ctxTp", bufs=2)
            zTp = pool(name="zTp", bufs=1)
            xp = pool(name="xp", bufs=1)
            yp = pool(name="yp", bufs=1)
            zp = pool(name="zp", bufs=1)
            tbp = pool(name="tb", bufs=3)
            yqp = pool(name="yq", bufs=2)
            outp = pool(name="outp", bufs=2)
            rwp = pool(name="rw", bufs=4)
            rcp = pool(name="rc", bufs=4)
            bigp = pool(name="ps1024", bufs=3, space="PSUM")
            mmp = pool(name="mm512", bufs=2, space="PSUM")
            _es.enter_context(nc.allow_low_precision(
                reason="bf16 LN stats are within tolerance; residual dominates"))

            # ---------------- DMAs, earliest-needed first -----------------
            cond8_sb = [None] * BL
            gn8_sb = [None] * BL
            gnT_sb = [None] * BL
            cond8_sb[0] = cond8p.tile([P, KL, NK], F8, tag="cond8", name="cond8_0")
            nc.sync.dma_start(cond8_sb[0][:], cond8[0].rearrange("(ko p) t -> p ko t", p=P))
            kWe8_sb = cst.tile([P, KL, D], F8, tag="kWe8")
            nc.sync.dma_start(kWe8_sb[:], kWe8[:].rearrange("(ko p) n -> p ko n", p=P))
            gn8_sb[0] = gn8p.tile([P, KD, NQ], F8, tag="gn8", name="gn8_0")
            nc.sync.dma_start(gn8_sb[0][:], gn8[0].rearrange("(ko p) t -> p ko t", p=P))
            qWe8_sb = cst.tile([P, KD, D], F8, tag="qWe8")
            nc.sync.dma_start(qWe8_sb[:], qWe8[:].rearrange("(ko p) n -> p ko n", p=P))
            vWe8_sb = cst.tile([P, KL, D], F8, tag="vWe8")
            nc.sync.dma_start(vWe8_sb[:], vWe8[:].rearrange("(ko p) n -> p ko n", p=P))
            vld_sb = cst.tile([P, BL * MK], F32, tag="vld")
            nc.sync.dma_start(vld_sb[:], vld[:])
            vld8_sb = cst.tile([P, BL * MK, 1], F8, tag="vld8")
            nc.sync.dma_start(vld8_sb[:], vld8[:])
            onesr_sb = cst.tile([1, P], BF16, tag="onesr")
            nc.sync.dma_start(onesr_sb[:], onesr[:])
            onesc_sb = cst.tile([P, 1], BF16, tag="onesc")
            nc.sync.dma_start(onesc_sb[:], onesc[:])
            bc_sb = cst.tile([P, 8 * KD], F32, tag="bcols")
            nc.sync.dma_start(bc_sb[:], bcols[:])
            vber_sb = cst.tile([1, D], BF16, tag="vber")
            nc.sync.dma_start(vber_sb[:], vber[:])
            cond8_sb[1] = cond8p.tile([P, KL, NK], F8, tag="cond8", name="cond8_1")
            nc.sync.dma_start(cond8_sb[1][:], cond8[1].rearrange("(ko p) t -> p ko t", p=P))
            gn8_sb[1] = gn8p.tile([P, KD, NQ], F8, tag="gn8", name="gn8_1")
            nc.sync.dma_start(gn8_sb[1][:], gn8[1].rearrange("(ko p) t -> p ko t", p=P))
            gnT_sb[0] = gnTp.tile([P, KD, NQ], BF16, tag="gnT", name="gnT_0")
            nc.sync.dma_start(gnT_sb[0][:], gnT[0].rearrange("(ko p) t -> p ko t", p=P))
            outW8_sb = cst.tile([P, KD, D], F8, tag="outW8")
            nc.sync.dma_start(outW8_sb[:], outW8[:].rearrange("(ko p) n -> p ko n", p=P))
            d1W_sb = cst.tile([P, KD, D], BF16, tag="d1W")
            nc.sync.dma_start(d1W_sb[:], d1W[:].rearrange("(ko p) n -> p ko n", p=P))
            gnT_sb[1] = gnTp.tile([P, KD, NQ], BF16, tag="gnT", name="gnT_1")
            nc.sync.dma_start(gnT_sb[1][:], gnT[1].rearrange("(ko p) t -> p ko t", p=P))

            qbe_c = bc_sb[:, 0 * KD:1 * KD]
            kbe_c = bc_sb[:, 1 * KD:2 * KD]
            outb_c = bc_sb[:, 2 * KD:3 * KD]
            d1b_c = bc_sb[:, 3 * KD:4 * KD]
            ln1g_c = bc_sb[:, 4 * KD:5 * KD]
            ln1b_c = bc_sb[:, 5 * KD:6 * KD]
            ln2g_c = bc_sb[:, 6 * KD:7 * KD]
            ln2b_c = bc_sb[:, 7 * KD:8 * KD]

            qhT = [None] * BL
            khT = [None] * BL
            vh8 = [None] * BL
            ctok = [None] * BL
            ctxTb = [None] * BL
            ctxT8 = [None] * BL
            pt8 = [[None] * HP for _ in range(BL)]

            # ------------- projections, per 128-feature chunk -------------
            def kq_chunk(b, m):
                if khT[b] is None:
                    khT[b] = khp.tile([P, KD, NK], BF16, tag="khT", name="khT")
                    qhT[b] = qhp.tile([P, KD, NQ], BF16, tag="qhT", name="qhT")
                ks = bigp.tile([P, NK], F32, tag="ps1024", name="ks")
                for n in range(2):
                    for i in range(KL // 2):
                        nc.tensor.matmul(ks[:, n * 512:(n + 1) * 512],
                                         kWe8_sb[:, 2 * i:2 * i + 2, m * P:(m + 1) * P],
                                         cond8_sb[b][:, 2 * i:2 * i + 2, n * 512:(n + 1) * 512],
                                         start=(i == 0), stop=(i == KL // 2 - 1),
                                         perf_mode=PM.DoubleRow)
                if trivial_affine:
                    nc.vector.tensor_scalar_mul(khT[b][:, m, :], ks[:, :NK], DESC_K)
                else:
                    nc.vector.tensor_scalar(khT[b][:, m, :], ks[:, :NK], DESC_K,
                                            kbe_c[:, m:m + 1], ALU.mult, ALU.add)
                qs = mmp.tile([P, 512], F32, tag="mm512", name="qs")
                for i in range(KD // 2):
                    nc.tensor.matmul(qs[:, :NQ], qWe8_sb[:, 2 * i:2 * i + 2, m * P:(m + 1) * P],
                                     gn8_sb[b][:, 2 * i:2 * i + 2, :],
                                     start=(i == 0), stop=(i == KD // 2 - 1),
                                     perf_mode=PM.DoubleRow)
                if trivial_affine:
                    nc.scalar.activation(qhT[b][:, m, :], qs[:, :NQ], AF.Copy, scale=DESC_Q)
                else:
                    nc.vector.tensor_scalar(qhT[b][:, m, :], qs[:, :NQ], DESC_Q,
                                            qbe_c[:, m:m + 1], ALU.mult, ALU.add)

            def vproj_all(b):
                vh8[b] = vhp.tile([P, MK, D], F8, tag="vh8", name="vh8")
                for mk in range(MK):
                    vs = bigp.tile([P, NK], F32, tag="ps1024", name="vs")
                    for (n0, nw) in ((0, 512), (512, 256)):
                        for i in range(KL // 2):
                            nc.tensor.matmul(vs[:, n0:n0 + nw],
                                             cond8_sb[b][:, 2 * i:2 * i + 2, mk * P:(mk + 1) * P],
                                             vWe8_sb[:, 2 * i:2 * i + 2, n0:n0 + nw],
                                             start=(i == 0),
                                             stop=(i == KL // 2 - 1 and trivial_affine),
                                             perf_mode=PM.DoubleRow)
                    if not trivial_affine:
                        for (n0, nw) in ((0, 512), (512, 256)):
                            nc.tensor.matmul(vs[:, n0:n0 + nw], onesr_sb[0:1, 0:P],
                                             vber_sb[:, n0:n0 + nw], start=False, stop=True,
                                             skip_group_check=True)
                    if mk % 2 == 0:
                        nc.vector.tensor_scalar_mul(vh8[b][:, mk, :], vs[:, :D],
                                                    vld_sb[:, b * MK + mk:b * MK + mk + 1])
                    else:
                        nc.scalar.activation(vh8[b][:, mk, :], vs[:, :D], AF.Copy,
                                             scale=vld_sb[:, b * MK + mk:b * MK + mk + 1])

            # ------------------------ attention ---------------------------
            def s_exp(b, hp):
                pt8[b][hp] = [ptp.tile([P, MK, NQ], F8, tag="pt8", name=f"pt{e}")
                              for e in range(2)]
                for g2 in range(MK // 2):
                    sps = [bigp.tile([P, NK], F32, tag="ps1024", name=f"s{e}")
                           for e in range(2)]
                    for half in range(2):
                        mk = 2 * g2 + half
                        for e in range(2):
                            off = DH * e
                            nc.tensor.matmul(sps[e][:, half * NQ:(half + 1) * NQ],
                                             khT[b][off:off + DH, hp, mk * P:(mk + 1) * P],
                                             qhT[b][off:off + DH, hp, :],
                                             start=True, stop=True)
                    for e in range(2):
                        nc.scalar.activation(pt8[b][hp][e][:, 2 * g2:2 * g2 + 2, :],
                                             sps[e][:], AF.Exp)

            def ctx_pair(b, hp):
                if ctok[b] is None:
                    ctok[b] = ctokp.tile([P, HP, MQ, 2 * DH], BF16, tag="ctok", name="ctok")
                    ctxTb[b] = ctxTbp.tile([P, KD, MQ, P], BF16, tag="ctxTb", name="ctxTb")
                dn = bigp.tile([P, NK], F32, tag="ps1024", name="dn")
                cps = [None, None]
                for e in range(2):
                    h = 2 * hp + e
                    pt = pt8[b][hp][e]
                    cps[e] = mmp.tile([P, 512], F32, tag="mm512", name=f"cps{e}")
                    for mq in range(MQ):
                        for i in range(MK // 2):
                            nc.tensor.matmul(cps[e][:, mq * DH:(mq + 1) * DH],
                                             pt[:, 2 * i:2 * i + 2, mq * P:(mq + 1) * P],
                                             vh8[b][:, 2 * i:2 * i + 2, h * DH:(h + 1) * DH],
                                             start=(i == 0), stop=(i == MK // 2 - 1),
                                             perf_mode=PM.DoubleRow)
                        for i in range(MK // 2):
                            nc.tensor.matmul(dn[:, mq * 2 + e:mq * 2 + e + 1],
                                             pt[:, 2 * i:2 * i + 2, mq * P:(mq + 1) * P],
                                             vld8_sb[:, b * MK + 2 * i:b * MK + 2 * i + 2, :],
                                             start=(i == 0), stop=(i == MK // 2 - 1),
                                             perf_mode=PM.DoubleRow)
                rc = rcp.tile([P, 2 * MQ], F32, tag="rc")
                nc.vector.reciprocal(rc[:], dn[:, 0:2 * MQ])
                for mq in range(MQ):
                    for e in range(2):
                        nc.vector.tensor_scalar_mul(
                            ctok[b][:, hp, mq, e * DH:(e + 1) * DH],
                            cps[e][:, mq * DH:(mq + 1) * DH],
                            rc[:, mq * 2 + e:mq * 2 + e + 1])
                nc.sync.dma_start_transpose(ctxTb[b][:, hp], ctok[b][:, hp])

            # ------------------------ phase C -----------------------------
            def phase_C(b):
                ctxT8[b] = ctxTp.tile([P, KD, NQ], F8, tag="ctxT8", name="ctxT8")
                for m in range(KD):
                    nc.gpsimd.tensor_copy(ctxT8[b][:, m, :], ctxTb[b][:, m])
                x = xp.tile([P, KD, NQ], BF16, tag="x")
                for m in range(KD):
                    om = mmp.tile([P, 512], F32, tag="mm512", name="om")
                    for i in range(KD // 2):
                        nc.tensor.matmul(om[:, :NQ], outW8_sb[:, 2 * i:2 * i + 2, m * P:(m + 1) * P],
                                         ctxT8[b][:, 2 * i:2 * i + 2, :],
                                         start=(i == 0), stop=(i == KD // 2 - 1),
                                         perf_mode=PM.DoubleRow)
                    if trivial_affine:
                        nc.vector.scalar_tensor_tensor(x[:, m, :], om[:, :NQ], DESC_O,
                                                       gnT_sb[b][:, m, :], ALU.mult, ALU.add)
                    else:
                        t0 = tbp.tile([P, NQ], BF16, tag="tb", name="t0")
                        nc.vector.tensor_scalar(t0[:], om[:, :NQ], DESC_O,
                                                outb_c[:, m:m + 1], ALU.mult, ALU.add)
                        nc.gpsimd.tensor_tensor(x[:, m, :], t0[:], gnT_sb[b][:, m, :], ALU.add)
                # LN1
                ms = mmp.tile([P, 512], F32, tag="mm512", name="ms")
                for m in range(KD):
                    nc.tensor.matmul(ms[0:1, :NQ], onesc_sb[:], x[:, m, :],
                                     start=(m == 0), stop=(m == KD - 1))
                m_sb = rwp.tile([1, NQ], BF16, tag="rw", name="m_sb")
                nc.vector.tensor_scalar_mul(m_sb[:], ms[0:1, :NQ], 1.0 / D)
                if not trivial_affine:
                    sqs = mmp.tile([P, 512], F32, tag="mm512", name="sqs")
                    for m in range(KD):
                        xq = yqp.tile([P, NQ], BF16, tag="yq", name="xq")
                        nc.vector.tensor_tensor(xq[:], x[:, m, :], x[:, m, :], ALU.mult)
                        nc.tensor.matmul(sqs[0:1, :NQ], onesc_sb[:], xq[:],
                                         start=(m == 0), stop=(m == KD - 1))
                    e2 = rwp.tile([1, NQ], F32, tag="rw", name="e2")
                    nc.vector.tensor_scalar(e2[:], sqs[0:1, :NQ], 1.0 / D, EPS, ALU.mult, ALU.add)
                    mf = rwp.tile([1, NQ], F32, tag="rw", name="mf")
                    nc.vector.tensor_copy(mf[:], m_sb[:])
                    nc.vector.tensor_tensor(mf[:], mf[:], mf[:], ALU.mult)
                    nc.vector.tensor_tensor(e2[:], e2[:], mf[:], ALU.subtract)
                    sd = rwp.tile([1, NQ], F32, tag="rw", name="sd")
                    nc.scalar.activation(sd[:], e2[:], AF.Sqrt)
                    rs = rwp.tile([1, NQ], BF16, tag="rw", name="rs")
                    nc.vector.reciprocal(rs[:], sd[:])
                mb = mmp.tile([P, 512], F32, tag="mm512", name="mb")
                nc.tensor.matmul(mb[:, :NQ], onesr_sb[:], m_sb[:], start=True, stop=True)
                mb_s = tbp.tile([P, NQ], BF16, tag="tb", name="mb_s")
                nc.vector.tensor_copy(mb_s[:], mb[:, :NQ])
                u = x
                if trivial_affine:
                    for m in range(KD):
                        nc.vector.tensor_tensor(u[:, m, :], x[:, m, :], mb_s[:], ALU.subtract)
                else:
                    rb = mmp.tile([P, 512], F32, tag="mm512", name="rb")
                    nc.tensor.matmul(rb[:, :NQ], onesr_sb[:], rs[:], start=True, stop=True)
                    for m in range(KD):
                        nc.vector.tensor_tensor(u[:, m, :], x[:, m, :], mb_s[:], ALU.subtract)
                        nc.vector.tensor_tensor(u[:, m, :], u[:, m, :], rb[:, :NQ], ALU.mult)
                        nc.vector.tensor_scalar(u[:, m, :], u[:, m, :], ln1g_c[:, m:m + 1],
                                                ln1b_c[:, m:m + 1], ALU.mult, ALU.add)
                # FFN
                y = yp.tile([P, KD, NQ], BF16, tag="y")
                for m in range(KD):
                    fm = mmp.tile([P, 512], F32, tag="mm512", name="fm")
                    for kc in range(KD):
                        nc.tensor.matmul(fm[:, :NQ], d1W_sb[:, kc, m * P:(m + 1) * P],
                                         u[:, kc, :], start=(kc == 0), stop=(kc == KD - 1))
                    t1 = tbp.tile([P, NQ], BF16, tag="tb", name="t1")
                    nc.scalar.activation(t1[:], fm[:, :NQ], AF.Lrelu,
                                         bias=d1b_c[:, m:m + 1], alpha=0.01)
                    nc.gpsimd.tensor_tensor(y[:, m, :], t1[:], u[:, m, :], ALU.add)
                # LN2
                s2 = mmp.tile([P, 512], F32, tag="mm512", name="s2")
                sq2 = mmp.tile([P, 512], F32, tag="mm512", name="sq2")
                for m in range(KD):
                    nc.tensor.matmul(s2[0:1, :NQ], onesc_sb[:], y[:, m, :],
                                     start=(m == 0), stop=(m == KD - 1))
                    yq = yqp.tile([P, NQ], BF16, tag="yq", name="yq")
                    nc.vector.tensor_tensor(yq[:], y[:, m, :], y[:, m, :], ALU.mult)
                    nc.tensor.matmul(sq2[0:1, :NQ], onesc_sb[:], yq[:],
                                     start=(m == 0), stop=(m == KD - 1))
                m2 = rwp.tile([1, NQ], BF16, tag="rw", name="m2")
                nc.vector.tensor_scalar_mul(m2[:], s2[0:1, :NQ], 1.0 / D)
                e2b = rwp.tile([1, NQ], F32, tag="rw", name="e2b")
                nc.vector.tensor_scalar(e2b[:], sq2[0:1, :NQ], 1.0 / D, EPS, ALU.mult, ALU.add)
                m2f = rwp.tile([1, NQ], F32, tag="rw", name="m2f")
                nc.vector.tensor_copy(m2f[:], m2[:])
                nc.vector.tensor_tensor(m2f[:], m2f[:], m2f[:], ALU.mult)
                nc.vector.tensor_tensor(e2b[:], e2b[:], m2f[:], ALU.subtract)
                sd2 = rwp.tile([1, NQ], F32, tag="rw", name="sd2")
                nc.scalar.activation(sd2[:], e2b[:], AF.Sqrt)
                rs2 = rwp.tile([1, NQ], BF16, tag="rw", name="rs2")
                nc.vector.reciprocal(rs2[:], sd2[:])
                m2b = mmp.tile([P, 512], F32, tag="mm512", name="m2b")
                nc.tensor.matmul(m2b[:, :NQ], onesr_sb[:], m2[:], start=True, stop=True)
                r2b = mmp.tile([P, 512], F32, tag="mm512", name="r2b")
                nc.tensor.matmul(r2b[:, :NQ], onesr_sb[:], rs2[:], start=True, stop=True)
                m2b_s = tbp.tile([P, NQ], BF16, tag="tb", name="m2b_s")
                nc.vector.tensor_copy(m2b_s[:], m2b[:, :NQ])
                r2b_s = tbp.tile([P, NQ], BF16, tag="tb", name="r2b_s")
                nc.vector.tensor_copy(r2b_s[:], r2b[:, :NQ])
                z = zp.tile([P, KD, NQ], BF16, tag="z")
                for m in range(KD):
                    zt = tbp.tile([P, NQ], BF16, tag="tb", name="zt")
                    nc.gpsimd.tensor_tensor(zt[:], y[:, m, :], m2b_s[:], ALU.subtract)
                    nc.vector.tensor_tensor(z[:, m, :], zt[:], r2b_s[:], ALU.mult)
                    if not trivial_affine:
                        nc.vector.tensor_scalar(z[:, m, :], z[:, m, :], ln2g_c[:, m:m + 1],
                                                ln2b_c[:, m:m + 1], ALU.mult, ALU.add)
                zT = zTp.tile([P, MQ, D], BF16, tag="zT", name="zT")
                for m in range(KD):
                    nc.sync.dma_start_transpose(zT[:, :, m * P:(m + 1) * P], z[:, m, :])
                for t in range(MQ):
                    o_sb = outp.tile([P, D], F32, tag="outp")
                    nc.gpsimd.tensor_copy(o_sb[:], zT[:, t, :])
                    nc.sync.dma_start(out[b, t * P:(t + 1) * P, :], o_sb[:])

            # ---------------- schedule ------------------------------------
            for m in range(HP - 1):
                kq_chunk(0, m)
                s_exp(0, m)
            vproj_all(0)
            ctx_pair(0, 0)
            kq_chunk(0, HP - 1)
            s_exp(0, HP - 1)
            for hp in range(1, HP):
                ctx_pair(0, hp)
                kq_chunk(1, hp - 1)
                s_exp(1, hp - 1)
            vproj_all(1)
            ctx_pair(1, 0)
            kq_chunk(1, HP - 1)
            s_exp(1, HP - 1)
            phase_C(0)
            for hp in range(1, HP):
                ctx_pair(1, hp)
            phase_C(1)

    nc.compile()
    return nc


def kernel(**inputs):
    gn = np.asarray(inputs["graph_nodes"], dtype=np.float32)
    cond = np.asarray(inputs["conditioning_vector"], dtype=np.float32)
    mask = np.asarray(inputs["conditioning_attention_mask"])
    g = lambda k: np.asarray(inputs[k], dtype=np.float32)

    qW, qb = g("qW"), g("qb")
    kW, kb = g("kW"), g("kb")
    vW, vb = g("vW"), g("vb")
    in_qW, in_qb = g("in_qW"), g("in_qb")
    in_kW, in_kb = g("in_kW"), g("in_kb")
    in_vW, in_vb = g("in_vW"), g("in_vb")
    outW, outb = g("outW"), g("outb")
    ln1g, ln1b = g("ln1g"), g("ln1b")
    d1W, d1b = g("d1W"), g("d1b")
    ln2g, ln2b = g("ln2g"), g("ln2b")

    scale = 1.0 / np.sqrt(np.float32(DH))
    qWe = (qW @ in_qW) * scale
    qbe = (qb @ in_qW + in_qb) * scale
    kWe = kW @ in_kW
    kbe = kb @ in_kW + in_kb
    vWe = vW @ in_vW
    vbe = vb @ in_vW + in_vb

    trivial = bool(
        np.all(qbe == 0) and np.all(kbe == 0) and np.all(vbe == 0)
        and np.all(outb == 0) and np.all(d1b == 0)
        and np.all(ln1g == 1) and np.all(ln1b == 0)
        and np.all(ln2g == 1) and np.all(ln2b == 0))

    col = lambda v: np.ascontiguousarray(v.reshape(KD, P).T, dtype=np.float32)
    bcols = np.concatenate(
        [col(qbe), col(kbe), col(outb), col(d1b),
         col(ln1g), col(ln1b), col(ln2g), col(ln2b)], axis=1)

    valid01 = np.where(mask, 0.0, 1.0).astype(np.float32)

    key = ("nc", trivial)
    if key not in _NC_CACHE:
        _NC_CACHE[key] = _build_nc(trivial)
        _NC_CACHE["nc"] = _NC_CACHE[key]
    nc = _NC_CACHE[key]

    f8 = lambda a: np.ascontiguousarray(np.clip(a, -448, 448).astype(NPF8))
    bf = lambda a: np.ascontiguousarray(a.astype(NPBF))
    shared = {
        "qWe8": f8(qWe * SQ), "kWe8": f8(kWe * SK), "vWe8": f8(vWe * SV),
        "outW8": f8(outW * SO), "d1W": bf(d1W),
        "bcols": np.ascontiguousarray(bcols),
        "vber": bf((SV * vbe)[None, :]),
        "onesr": np.ones((1, P), NPBF),
        "onesc": np.ones((P, 1), NPBF),
    }
    in_maps = []
    for c in range(NCORES):
        bs = slice(c * BL, (c + 1) * BL)
        vp = np.zeros((P, BL * MK), np.float32)
        for i, bb in enumerate(range(c * BL, (c + 1) * BL)):
            vp[:, i * MK:(i + 1) * MK] = valid01[bb].reshape(MK, P).T
        in_maps.append({
            **shared,
            "gn8": f8(gn[bs].transpose(0, 2, 1)),
            "gnT": bf(gn[bs].transpose(0, 2, 1)),
            "cond8": f8(cond[bs].transpose(0, 2, 1)),
            "vld": vp,
            "vld8": f8((vp * 0.25)[:, :, None]),
        })

    res = run_bass_kernel_spmd(nc, in_maps, list(range(NCORES)))
    return np.concatenate([res.results[c]["out"] for c in range(NCORES)], axis=0)
